# revision 7
# baseline (speedup 1.0000x reference)
"""Trainium2 Bass kernel v3: ablation-style attention (nn_Attention).

Sharding: 8 cores = 4 batches x 2 residual streams. Each core computes one
stream's full causal attention (1024 queries x 1024 keys, 12 heads) and the
12 per-head output projections:
  o12[k] = z_k @ W_O_k        [12, 1024, 768] f16  (bias-free)
Host unshards linearly (the tensor-parallel gather):
  ch_p      = sum_k o12_p[k] + b_O
  out[0/1]  = ch0 / ch1
  out[2+k]  = ch1 - o1_k + o0_k
All model matmuls stay on device; host does only shard recombination.

Causal: only the 36 valid (key-tile, query-range) pairs are computed per
head (12 chunks/head-pair). Diagonal tiles get a multiplicative triangular
mask on the P tiles (one strided DVE op covers both heads).

Pipeline per head-pair g: proj(g+1) | S/exp/AV(g) | norm+head-out(g-1),
with per-head outputs DMA'd continuously. PSUM: banks0-5 = unified
rotating pool (S chunks, projections, norm broadcast, head-out units),
banks6-7 = AV accumulators (per head, per query-half).
"""

import os
import numpy as np
import ml_dtypes

N_HEADS = 12
D_MODEL = 768
D_HEAD = 64
B = 4
S = 1024
NT_D = 6
BF16 = ml_dtypes.bfloat16

# (key_tile, q_offset, n_cols); lo-half chunks first, then hi-half
CHUNKS = [
    (0, 0, 512), (1, 128, 384), (2, 256, 256), (3, 384, 128),
    (0, 512, 512), (1, 512, 512), (2, 512, 512), (3, 512, 512),
    (4, 512, 512), (5, 640, 384), (6, 768, 256), (7, 896, 128),
]
DIAG = {0, 1, 2, 3, 8, 9, 10, 11}
NPT = 4
NOS = 4
LAST_EXEC_NS = None
_GRAPH = None


def _build_graph():
    import concourse.bass as bass
    import concourse.mybir as mybir
    from contextlib import ExitStack

    f32 = mybir.dt.float32
    bf16 = mybir.dt.bfloat16
    f16 = mybir.dt.float16
    Exp = mybir.ActivationFunctionType.Exp
    Ident = mybir.ActivationFunctionType.Identity

    nc = bass.Bass()

    xt_d = nc.declare_dram_parameter("xt", [128, NT_D, S], bf16, isOutput=False)
    wq_d = nc.declare_dram_parameter("wq", [128, NT_D, 768], bf16, isOutput=False)
    wk_d = nc.declare_dram_parameter("wk", [128, NT_D, 768], bf16, isOutput=False)
    wv_d = nc.declare_dram_parameter("wv", [128, NT_D, 780], bf16, isOutput=False)
    wo_d = nc.declare_dram_parameter("wo", [128, NT_D, 768], bf16, isOutput=False)
    bq_d = nc.declare_dram_parameter("bq", [128, NT_D], f32, isOutput=False)
    bk_d = nc.declare_dram_parameter("bk", [128, NT_D], f32, isOutput=False)
    vb_d = nc.declare_dram_parameter("vb", [1, 780], bf16, isOutput=False)
    tri2_d = nc.declare_dram_parameter("tri2", [128, 256], bf16, isOutput=False)
    id16_d = nc.declare_dram_parameter("id16", [16, 16], f32, isOutput=False)
    o12_d = nc.declare_dram_parameter("o12", [12, 8, 128, 768], f16, isOutput=True)

    ctx = ExitStack()
    sb = lambda name, shape, dt: ctx.enter_context(nc.sbuf_tensor(name, shape, dt))

    xt = sb("xt_s", [128, NT_D, S], bf16)
    wq = sb("wq_s", [128, NT_D, 768], bf16)
    wk = sb("wk_s", [128, NT_D, 768], bf16)
    wv = sb("wv_s", [128, NT_D, 780], bf16)
    wo = sb("wo_s", [128, NT_D, 768], bf16)
    bq = sb("bq_s", [128, NT_D], f32)
    bk = sb("bk_s", [128, NT_D], f32)
    vb = sb("vb_s", [1, 780], bf16)
    tri2 = sb("tri2_s", [128, 256], bf16)
    id16 = sb("id16_s", [16, 16], f32)
    ones_b = sb("ones_b", [1, S], bf16)

    qT = sb("qT", [128, NT_D, S], bf16)
    kT = sb("kT", [128, NT_D, S], bf16)
    vA = sb("vA", [128, 8, 780], bf16)
    zT = sb("zT", [128, NT_D, S], bf16)
    pts = [sb(f"pt{i}", [128, 1024], bf16) for i in range(NPT)]
    den_sb = sb("den_sb", [1, 12 * S], f32)
    den96 = sb("den96", [16, 768], f32)
    recipT = sb("recipT", [128, 96], f32)
    ostage = [sb(f"ostage{i}", [128, 768], f16) for i in range(NOS)]

    ps = ctx.enter_context(nc.psum_tensor("psP", [128, 3072], f32))
    psZ = [ctx.enter_context(nc.psum_tensor(f"psZ{i}", [128, 512], f32))
           for i in range(2)]

    class Ctr:
        __slots__ = ("sem", "n")

        def __init__(self, name):
            self.sem = ctx.enter_context(nc.semaphore(name))
            self.n = 0

    G = [Ctr(f"g{i}") for i in range(5)]
    PEc = Ctr("pe")
    ACTc = Ctr("act")
    DVEc = Ctr("dve")
    CH = [Ctr(f"ch{i}") for i in range(NOS)]
    DN = Ctr("dn")

    prog = {k: [] for k in ("pe", "act", "dve", "sync")}
    observed = {k: {} for k in prog}

    def op(eng, fn):
        prog[eng].append(fn)

    def wait(eng, ctr, val):
        if val is None or val <= 0:
            return
        key = id(ctr)
        if observed[eng].get(key, 0) >= val:
            return
        observed[eng][key] = val
        op(eng, lambda e, s=ctr.sem, v=val: e.wait_ge(s, v))

    def emit(eng, build, inc=None, k=1):
        ev = None
        if inc is not None:
            inc.n += k
            ev = inc.n

        def f(e, b=build, i=inc, kk=k):
            r = b(e)
            if i is not None:
                r.then_inc(i.sem, kk)

        op(eng, f)
        return ev

    # ---- constants / warmup ----
    ev_ones = emit("dve", lambda e: e.memset(ones_b[:], 1.0), inc=DVEc)
    wait("act", DVEc, ev_ones)
    emit("act", lambda e: e.activation(
        pts[0][0:1, 0:1], ones_b[0:1, 0:1], Exp, bias=0.0, scale=1.0), inc=ACTc)

    # ---- input DMAs ----
    loads = [
        (xt[:, 0:2], xt_d[:, 0:2], 0), (xt[:, 2:4], xt_d[:, 2:4], 0),
        (xt[:, 4:6], xt_d[:, 4:6], 0),
        (wq[:, 0:3], wq_d[:, 0:3], 0), (wq[:, 3:6], wq_d[:, 3:6], 0),
        (bq[:], bq_d[:], 0),
        (wk[:, 0:3], wk_d[:, 0:3], 1), (wk[:, 3:6], wk_d[:, 3:6], 1),
        (bk[:], bk_d[:], 1),
        (wv[:], wv_d[:], 2), (vb[:], vb_d[:], 2),
        (tri2[:], tri2_d[:], 3), (id16[:], id16_d[:], 3),
        (wo[:], wo_d[:], 4),
    ]
    gtot = [0] * 5
    for a_, b_, gi in loads:
        gtot[gi] += 16
    for a_, b_, gi in loads:
        emit("sync", lambda e, a=a_, b=b_: e.dma_start(out=a, in_=b),
             inc=G[gi], k=16)

    # ================= unified psum pool (banks 0-5 of ps) =================
    bank_war = [None] * 6     # (ctr, val) that frees the bank
    bank_cur = [0]

    def alloc_banks(n):
        # n = 1 or 2 (pair must be even-aligned)
        cur = bank_cur[0]
        if n == 2 and cur % 2 == 1:
            cur += 1
        cur = cur % 6
        banks = [cur, cur + 1] if n == 2 else [cur]
        bank_cur[0] = (banks[-1] + 1) % 6
        for b_ in banks:
            w = bank_war[b_]
            if w is not None:
                wait("pe", w[0], w[1])
        return banks

    def set_war(banks, ctr, val):
        for b_ in banks:
            bank_war[b_] = (ctr, val)

    qk_ready = {}
    v_ready = {}
    exp_ev = {}
    pt_rdy = {}               # chunk u -> (ctr, val) gating AV read of pt
    av_ev = {}
    psz_war = [None, None]    # per head-slot: (ctr, val) of last evac
    rt_ev = {}                # g -> DVE event: recipT cols for pair g ready
    den_dma = {}
    os_i = [0]
    ho_n = [0]

    # ---------------- projection units ----------------
    def proj_q_unit(gp, half, which):
        w_sb, b_sb, dst = (wq, bq, qT) if which == 'q' else (wk, bk, kT)
        bks = alloc_banks(1)
        off = bks[0] * 512
        wait("pe", G[0], gtot[0])
        if which == 'k':
            wait("pe", G[1], gtot[1])
        ev = None
        for dt in range(NT_D):
            ev = emit("pe", lambda e, o=ps[:, off:off + 512],
                      l=w_sb[:, dt, gp * 128:(gp + 1) * 128],
                      r=xt[:, dt, half * 512:(half + 1) * 512],
                      s=(dt == 0), st_=(dt == NT_D - 1):
                      e.matmul(o, l, r, start=s, stop=st_),
                      inc=PEc if dt == NT_D - 1 else None)
        wait("dve", PEc, ev)
        cev = emit("dve", lambda e, o=dst[:, gp, half * 512:(half + 1) * 512],
                   i=ps[:, off:off + 512], bb=b_sb[:, gp:gp + 1]:
                   e.tensor_scalar_add(o, i, bb), inc=DVEc)
        set_war(bks, DVEc, cev)
        qk_ready[gp] = cev

    def proj_v_unit(pt3, sp):
        # v columns for head-pairs 3*pt3..3*pt3+2 (390 cols), key-tiles 2sp,2sp+1
        bks = alloc_banks(2)
        off = bks[0] * 512
        wait("pe", G[0], gtot[0])
        wait("pe", G[2], gtot[2])
        sts = [2 * sp, 2 * sp + 1]
        ev = None
        for si, st in enumerate(sts):
            o_ap = ps[:, off + 512 * si:off + 512 * si + 390]
            for dt in range(NT_D):
                emit("pe", lambda e, o=o_ap,
                     l=xt[:, dt, st * 128:(st + 1) * 128],
                     r=wv[:, dt, 390 * pt3:390 * pt3 + 390], s=(dt == 0):
                     e.matmul(o, l, r, start=s, stop=False))
            ev = emit("pe", lambda e, o=o_ap,
                      l=ones_b[0:1, 0:128], r=vb[0:1, 390 * pt3:390 * pt3 + 390]:
                      e.matmul(o, l, r, start=False, stop=True),
                      inc=PEc if si == 1 else None)
        wait("dve", PEc, ev)
        src = ps[:, off:off + 1024].rearrange("p (n f) -> p n f", n=2)[:, :, 0:390]
        dst = vA[:, 2 * sp:2 * sp + 2, 390 * pt3:390 * pt3 + 390]
        cev = emit("dve", lambda e, o=dst, i=src: e.tensor_copy(o, i), inc=DVEc)
        set_war(bks, DVEc, cev)
        v_ready[pt3] = cev

    # ---------------- denominator reciprocal, transposed (pair g) ----------
    # recipT[:, 16g + 8*hs + mt] = 1/den[2g+hs, 128*mt + row]; the softmax
    # normalization commutes with W_O, so it's applied as a per-partition
    # scale during head-out evacuation.
    def recipT_chain(g):
        wait("pe", DN, den_dma[g])
        wait("pe", G[3], gtot[3])
        bks = alloc_banks(1)
        off = bks[0] * 512
        tev = emit("pe", lambda e, o=ps[:, off:off + 16],
                   i=den96[0:16, 128 * g:128 * g + 128], ii=id16[:, :]:
                   e.transpose(o, i, ii), inc=PEc)
        wait("dve", PEc, tev)

        def _recip(e, o=recipT[:, 16 * g:16 * g + 16], i=ps[:, off:off + 16]):
            with nc.allow_low_precision(reason="softmax denom recip"):
                return e.reciprocal(o, i)

        rev = emit("dve", _recip, inc=DVEc)
        set_war(bks, DVEc, rev)
        rt_ev[g] = rev

    # ---------------- head-out unit ----------------
    def ho_unit(g, j, tail=False):
        # interleave heads so consecutive units hit different PE row groups
        hs, mt = j % 2, j // 2
        h = 2 * g + hs
        po = hs * 64
        wait("pe", DVEc, rt_ev[g])
        wait("pe", G[4], gtot[4])
        bks = alloc_banks(2)
        off = bks[0] * 512
        emit("pe", lambda e, o=ps[:, off:off + 512],
             l=zT[po:po + 64, g, mt * 128:(mt + 1) * 128],
             r=wo[po:po + 64, g, 0:512]:
             e.matmul(o, l, r, start=True, stop=True))
        ev = emit("pe", lambda e, o=ps[:, off + 512:off + 768],
                  l=zT[po:po + 64, g, mt * 128:(mt + 1) * 128],
                  r=wo[po:po + 64, g, 512:768]:
                  e.matmul(o, l, r, start=True, stop=True), inc=PEc)
        c = os_i[0] % NOS
        os_i[0] += 1
        if tail:
            eng = "act" if j % 2 == 0 else "dve"
        else:
            eng = "act" if ho_n[0] % 3 == 1 else "dve"
        ho_n[0] += 1
        ctr = ACTc if eng == "act" else DVEc
        wait(eng, PEc, ev)
        wait(eng, CH[c], CH[c].n)
        sc = recipT[:, 16 * g + 8 * hs + mt:16 * g + 8 * hs + mt + 1]
        if eng == "act":
            wait(eng, DVEc, rt_ev[g])
            cev = emit(eng, lambda e, o=ostage[c][:, :], i=ps[:, off:off + 768],
                       s=sc: e.activation(o, i, Ident, scale=s), inc=ctr)
        else:
            cev = emit(eng, lambda e, o=ostage[c][:, :], i=ps[:, off:off + 768],
                       s=sc: e.tensor_scalar_mul(o, i, s), inc=ctr)
        set_war(bks, ctr, cev)
        wait("sync", ctr, cev)
        emit("sync", lambda e, o=o12_d[h, mt], i=ostage[c][:, :]:
             e.dma_start(out=o, in_=i), inc=CH[c], k=16)

    # ---------------- attention ----------------
    def s_chunk(g, i, u):
        kt, qoff, c = CHUNKS[i]
        bks = alloc_banks(2)
        off = bks[0] * 512
        wait("pe", DVEc, qk_ready[g])
        ev = None
        for hs in range(2):
            po = hs * 64
            ev = emit("pe", lambda e,
                      o=ps[:, off + hs * 512:off + hs * 512 + c],
                      l=kT[po:po + 64, g, kt * 128:(kt + 1) * 128],
                      r=qT[po:po + 64, g, qoff:qoff + c]:
                      e.matmul(o, l, r, start=True, stop=True),
                      inc=PEc if hs == 1 else None)
        slot = u % NPT
        w = pt_rdy.get(u - NPT)
        if w is not None:
            wait("act", w[0], w[1])
        wait("act", PEc, ev)
        src = ps[:, off:off + 1024].rearrange("p (n f) -> p n f", n=2)[:, :, 0:c]
        dst = pts[slot][:, 0:2 * c].rearrange("p (n f) -> p n f", n=2)
        eev = emit("act", lambda e, o=dst, i=src:
                   e.activation(o, i, Exp, bias=0.0, scale=0.125), inc=ACTc)
        exp_ev[u] = eev
        set_war(bks, ACTc, eev)
        if i in DIAG:
            # multiplicative triangular mask on both heads' diagonal 128-col
            # blocks: pt[:, {0:128, c:c+128}] *= tri
            wait("dve", G[3], gtot[3])
            wait("dve", ACTc, eev)
            ap = pts[slot][:, 0:2 * c].rearrange(
                "p (n f) -> p n f", n=2)[:, :, 0:128]
            mev = emit("dve", lambda e, o=ap,
                       m=tri2[:, :].rearrange("p (n f) -> p n f", n=2):
                       e.tensor_mul(o, o, m), inc=DVEc)
            pt_rdy[u] = (DVEc, mev)
        else:
            pt_rdy[u] = (ACTc, eev)

    def av_chunk(g, i, u):
        kt, qoff, c = CHUNKS[i]
        qo = qoff - 512 * (i >= 4)
        slot = u % NPT
        ctr, v = pt_rdy[u]
        wait("pe", ctr, v)
        wait("pe", DVEc, v_ready[g // 3])
        start = i in (0, 4)
        stop = i in (3, 11)
        ev = None
        for hs in range(2):
            h = 2 * g + hs
            if start and psz_war[hs] is not None:
                wait("pe", psz_war[hs][0], psz_war[hs][1])
            ev = emit("pe", lambda e, o=psZ[hs][0:65, qo:qo + c],
                      l=vA[:, kt, 65 * h:65 * h + 65],
                      r=pts[slot][:, hs * c:hs * c + c],
                      s=start, st_=stop:
                      e.matmul(o, l, r, start=s, stop=st_),
                      inc=PEc if hs == 1 else None)
        av_ev[u] = ev

    def evac_half(g, half, u_last):
        # head0 via DVE, head1 via ACT (parallel evacuation chains)
        for hs, eng, ctr in ((0, "dve", DVEc), (1, "act", ACTc)):
            h = 2 * g + hs
            po = hs * 64
            wait(eng, PEc, av_ev[u_last])
            zt_ap = zT[po:po + 64, g, half * 512:(half + 1) * 512]
            dn_ap = den_sb[0:1, 1024 * h + half * 512:
                           1024 * h + (half + 1) * 512]
            if eng == "dve":
                emit(eng, lambda e, o=zt_ap, i=psZ[hs][0:64, :]:
                     e.tensor_copy(o, i))
                dev = emit(eng, lambda e, o=dn_ap, i=psZ[hs][64:65, :]:
                           e.tensor_copy(o, i), inc=ctr)
            else:
                emit(eng, lambda e, o=zt_ap, i=psZ[hs][0:64, :]:
                     e.copy(o, i))
                dev = emit(eng, lambda e, o=dn_ap, i=psZ[hs][64:65, :]:
                           e.copy(o, i), inc=ctr)
            psz_war[hs] = (ctr, dev)
        if half == 1:
            for hs in range(2):
                h = 2 * g + hs
                wait("sync", psz_war[hs][0], psz_war[hs][1])
                emit("sync", lambda e,
                     o=den96[8 * hs:8 * hs + 8, 128 * g:128 * g + 128],
                     i=den_sb[0:1, 1024 * h:1024 * h + 1024]:
                     e.dma_start(out=o, in_=i), inc=DN, k=16)
            den_dma[g] = DN.n

    # ================= emission =================
    wait("pe", DVEc, ev_ones)
    # keep the PE activity monitor busy during the input-DMA window so the
    # clock gate is released before real work starts
    for _ in range(8):
        emit("pe", lambda e, o=psZ[0][:, 0:512],
             l=ones_b[0:1, 0:128], r=ones_b[0:1, 0:512]:
             e.matmul(o, l, r, start=True, stop=True))
    proj_q_unit(0, 0, 'q')
    proj_q_unit(0, 1, 'q')
    proj_q_unit(0, 0, 'k')
    proj_q_unit(0, 1, 'k')
    for sp in range(4):
        proj_v_unit(0, sp)

    def fillers_for(g):
        f = {i: [] for i in range(12)}
        if g < 5:
            gp = g + 1
            f[0].append(lambda: proj_q_unit(gp, 0, 'q'))
            f[1].append(lambda: proj_q_unit(gp, 1, 'q'))
            f[2].append(lambda: proj_q_unit(gp, 0, 'k'))
            f[3].append(lambda: proj_q_unit(gp, 1, 'k'))
        if g == 0:
            for sp in range(4):
                f[4 + 2 * sp].append(lambda sp=sp: proj_v_unit(1, sp))
        if g >= 1:
            gm = g - 1
            f[1].append(lambda: recipT_chain(gm))
            for j in range(16):
                f[2 + (j // 2)].append(lambda j=j: ho_unit(gm, j))
        return f

    for g in range(6):
        f = fillers_for(g)
        u0 = 12 * g
        s_chunk(g, 0, u0)
        s_chunk(g, 1, u0 + 1)
        for i in range(12):
            if i + 2 < 12:
                s_chunk(g, i + 2, u0 + i + 2)
            for th in f[i]:
                th()
            av_chunk(g, i, u0 + i)
            if i == 3:
                evac_half(g, 0, u0 + 3)
            if i == 11:
                evac_half(g, 1, u0 + 11)

    recipT_chain(5)
    for j in range(16):
        ho_unit(5, j, tail=True)

    for c in range(NOS):
        wait("sync", CH[c], CH[c].n)

    with nc.Block() as block:
        @block.tensor
        def _(e):
            for fn in prog["pe"]:
                fn(e)

        @block.scalar
        def _(e):
            for fn in prog["act"]:
                fn(e)

        @block.vector
        def _(e):
            for fn in prog["dve"]:
                fn(e)

        @block.sync
        def _(e):
            for fn in prog["sync"]:
                fn(e)

    ctx.close()
    return nc


def _prep_shared(inputs):
    WQ = np.asarray(inputs["W_Q"], np.float32)
    bQ = np.asarray(inputs["b_Q"], np.float32)
    WK = np.asarray(inputs["W_K"], np.float32)
    bK = np.asarray(inputs["b_K"], np.float32)
    WV = np.asarray(inputs["W_V"], np.float32)
    bV = np.asarray(inputs["b_V"], np.float32)
    WO = np.asarray(inputs["W_O"], np.float32)

    def to_tiles(w):
        return np.ascontiguousarray(
            w.reshape(NT_D, 128, w.shape[1]).transpose(1, 0, 2))

    wq = to_tiles(WQ.transpose(1, 0, 2).reshape(768, 768)).astype(BF16)
    wk = to_tiles(WK.transpose(1, 0, 2).reshape(768, 768)).astype(BF16)
    wv_aug = np.zeros((768, 780), np.float32)
    vb_row = np.zeros((1, 780), np.float32)
    for h in range(N_HEADS):
        wv_aug[:, 65 * h:65 * h + 64] = WV[h]
        vb_row[0, 65 * h:65 * h + 64] = bV[h]
        vb_row[0, 65 * h + 64] = 1.0
    wv = to_tiles(wv_aug).astype(BF16)
    wo = to_tiles(WO.reshape(768, 768)).astype(BF16)
    bq_r = np.ascontiguousarray(bQ.reshape(NT_D, 128).T).astype(np.float32)
    bk_r = np.ascontiguousarray(bK.reshape(NT_D, 128).T).astype(np.float32)

    r = np.arange(128)[:, None]
    c = np.arange(128)[None, :]
    tri = (r <= c).astype(np.float32)
    tri2 = np.concatenate([tri, tri], axis=1)

    return {
        "wq": wq, "wk": wk, "wv": wv, "wo": wo,
        "bq": bq_r, "bk": bk_r, "vb": vb_row.astype(BF16),
        "tri2": tri2.astype(BF16), "id16": np.eye(16, dtype=np.float32),
    }


def _prep_in_maps(inputs):
    nrp = np.asarray(inputs["normalized_resid_pre"], np.float32)
    alt = np.asarray(inputs["alt_normalized_resid_pre"], np.float32)
    shared = _prep_shared(inputs)
    in_maps = []
    for b in range(B):
        for p in range(2):
            x = nrp[b, 0] if p == 0 else alt[b]
            xt = np.ascontiguousarray(
                x.T.reshape(NT_D, 128, S).transpose(1, 0, 2))
            m = dict(shared)
            m["xt"] = xt.astype(BF16)
            in_maps.append(m)
    return in_maps


def _assemble(results, inputs):
    bO = np.asarray(inputs["b_O"], np.float32)
    out = np.empty((B, 14, S, D_MODEL), np.float32)
    for b in range(B):
        o0 = results[2 * b]["o12"].reshape(N_HEADS, S, D_MODEL).astype(np.float32)
        o1 = results[2 * b + 1]["o12"].reshape(N_HEADS, S, D_MODEL).astype(np.float32)
        ch0 = o0.sum(0) + bO
        ch1 = o1.sum(0) + bO
        out[b, 0] = ch0
        out[b, 1] = ch1
        out[b, 2:] = ch1[None] - o1 + o0
    return out


def _ensure_profile_hook():
    import sys
    import types

    try:
        from antenv.axon_hooks import get_axon_ntff_profile_hook  # noqa: F401
        return True
    except ImportError:
        pass
    try:
        from trn_agent_boot.trn_boot import _ntff_profile_via_ctypes

        hook = _ntff_profile_via_ctypes("/opt/axon/libaxon_pjrt.so")
        if hook is None:
            return False
        mod = types.ModuleType("antenv.axon_hooks")
        state = {"hook": hook}
        mod.set_axon_ntff_profile_hook = lambda h: state.update(hook=h)
        mod.get_axon_ntff_profile_hook = lambda: state["hook"]
        sys.modules["antenv.axon_hooks"] = mod
        import antenv

        antenv.axon_hooks = mod
        return True
    except Exception:
        return False


def kernel(**inputs):
    global LAST_EXEC_NS, _GRAPH
    from concourse.bass_utils import run_bass_kernel_spmd

    if _GRAPH is None:
        _GRAPH = _build_graph()
    nc = _GRAPH
    in_maps = _prep_in_maps(inputs)
    trace = os.environ.get("KERNEL_PROFILE", "0") == "1"
    if trace:
        trace = _ensure_profile_hook()
    res = run_bass_kernel_spmd(nc, in_maps, list(range(8)), trace=trace)
    LAST_EXEC_NS = res.exec_time_ns
    return _assemble(res.results, inputs)


# revision 8
# speedup vs baseline: 1.1825x; 1.1825x over previous
"""Trainium2 Bass kernel v3: ablation-style attention (nn_Attention).

Sharding: 8 cores = 4 batches x 2 residual streams. Each core computes one
stream's full causal attention (1024 queries x 1024 keys, 12 heads) and the
12 per-head output projections:
  o12[k] = z_k @ W_O_k        [12, 1024, 768] f16  (bias-free)
Host unshards linearly (the tensor-parallel gather):
  ch_p      = sum_k o12_p[k] + b_O
  out[0/1]  = ch0 / ch1
  out[2+k]  = ch1 - o1_k + o0_k
All model matmuls stay on device; host does only shard recombination.

Causal: only the 36 valid (key-tile, query-range) pairs are computed per
head (12 chunks/head-pair). Diagonal tiles get a multiplicative triangular
mask on the P tiles (one strided DVE op covers both heads).

Pipeline per head-pair g: proj(g+1) | S/exp/AV(g) | norm+head-out(g-1),
with per-head outputs DMA'd continuously. PSUM: banks0-5 = unified
rotating pool (S chunks, projections, norm broadcast, head-out units),
banks6-7 = AV accumulators (per head, per query-half).
"""

import os
import numpy as np
import ml_dtypes

N_HEADS = 12
D_MODEL = 768
D_HEAD = 64
B = 4
S = 1024
NT_D = 6
BF16 = ml_dtypes.bfloat16

# (key_tile, q_offset, n_cols); lo-half chunks first, then hi-half
CHUNKS = [
    (0, 0, 512), (1, 128, 384), (2, 256, 256), (3, 384, 128),
    (0, 512, 512), (1, 512, 512), (2, 512, 512), (3, 512, 512),
    (4, 512, 512), (5, 640, 384), (6, 768, 256), (7, 896, 128),
]
DIAG = {0, 1, 2, 3, 8, 9, 10, 11}
NPT = 4
NOS = 4
LAST_EXEC_NS = None
_GRAPH = None


def _build_graph():
    import concourse.bass as bass
    import concourse.mybir as mybir
    from contextlib import ExitStack

    f32 = mybir.dt.float32
    bf16 = mybir.dt.bfloat16
    f16 = mybir.dt.float16
    Exp = mybir.ActivationFunctionType.Exp
    Ident = mybir.ActivationFunctionType.Identity

    nc = bass.Bass()

    xt_d = nc.declare_dram_parameter("xt", [128, NT_D, S], bf16, isOutput=False)
    wq_d = nc.declare_dram_parameter("wq", [128, NT_D, 768], bf16, isOutput=False)
    wk_d = nc.declare_dram_parameter("wk", [128, NT_D, 768], bf16, isOutput=False)
    wv_d = nc.declare_dram_parameter("wv", [128, NT_D, 780], bf16, isOutput=False)
    wo_d = nc.declare_dram_parameter("wo", [128, NT_D, 768], bf16, isOutput=False)
    bq_d = nc.declare_dram_parameter("bq", [128, NT_D], f32, isOutput=False)
    bk_d = nc.declare_dram_parameter("bk", [128, NT_D], f32, isOutput=False)
    vb_d = nc.declare_dram_parameter("vb", [1, 780], bf16, isOutput=False)
    tri2_d = nc.declare_dram_parameter("tri2", [128, 256], bf16, isOutput=False)
    id16_d = nc.declare_dram_parameter("id16", [16, 16], f32, isOutput=False)
    o12_d = nc.declare_dram_parameter("o12", [12, 8, 128, 768], f16, isOutput=True)

    ctx = ExitStack()
    sb = lambda name, shape, dt: ctx.enter_context(nc.sbuf_tensor(name, shape, dt))

    xt = sb("xt_s", [128, NT_D, S], bf16)
    wq = sb("wq_s", [128, NT_D, 768], bf16)
    wk = sb("wk_s", [128, NT_D, 768], bf16)
    wv = sb("wv_s", [128, NT_D, 780], bf16)
    wo = sb("wo_s", [128, NT_D, 768], bf16)
    bq = sb("bq_s", [128, NT_D], f32)
    bk = sb("bk_s", [128, NT_D], f32)
    vb = sb("vb_s", [1, 780], bf16)
    tri2 = sb("tri2_s", [128, 256], bf16)
    id16 = sb("id16_s", [16, 16], f32)
    ones_b = sb("ones_b", [1, S], bf16)

    qT = sb("qT", [128, NT_D, S], bf16)
    kT = sb("kT", [128, NT_D, S], bf16)
    vA = sb("vA", [128, 8, 780], bf16)
    zT = sb("zT", [128, NT_D, S], bf16)
    pts = [sb(f"pt{i}", [128, 1024], bf16) for i in range(NPT)]
    den_sb = sb("den_sb", [1, 12 * S], f32)
    den96 = sb("den96", [16, 768], f32)
    recipT = sb("recipT", [128, 96], f32)
    ostage = [sb(f"ostage{i}", [128, 768], f16) for i in range(NOS)]

    ps = ctx.enter_context(nc.psum_tensor("psP", [128, 3072], f32))
    psZ = [ctx.enter_context(nc.psum_tensor(f"psZ{i}", [128, 512], f32))
           for i in range(2)]

    class Ctr:
        __slots__ = ("sem", "n")

        def __init__(self, name):
            self.sem = ctx.enter_context(nc.semaphore(name))
            self.n = 0

    G = [Ctr(f"g{i}") for i in range(5)]
    PEc = Ctr("pe")
    ACTc = Ctr("act")
    DVEc = Ctr("dve")
    CH = [Ctr(f"ch{i}") for i in range(NOS)]
    DN = Ctr("dn")

    prog = {k: [] for k in ("pe", "act", "dve", "sync")}
    observed = {k: {} for k in prog}

    def op(eng, fn):
        prog[eng].append(fn)

    def wait(eng, ctr, val):
        if val is None or val <= 0:
            return
        key = id(ctr)
        if observed[eng].get(key, 0) >= val:
            return
        observed[eng][key] = val
        op(eng, lambda e, s=ctr.sem, v=val: e.wait_ge(s, v))

    def emit(eng, build, inc=None, k=1):
        ev = None
        if inc is not None:
            inc.n += k
            ev = inc.n

        def f(e, b=build, i=inc, kk=k):
            r = b(e)
            if i is not None:
                r.then_inc(i.sem, kk)

        op(eng, f)
        return ev

    # ---- constants / warmup ----
    ev_ones = emit("dve", lambda e: e.memset(ones_b[:], 1.0), inc=DVEc)
    wait("act", DVEc, ev_ones)
    emit("act", lambda e: e.activation(
        pts[0][0:1, 0:1], ones_b[0:1, 0:1], Exp, bias=0.0, scale=1.0), inc=ACTc)

    # ---- input DMAs ----
    loads = [
        (xt[:, 0:2], xt_d[:, 0:2], 0), (xt[:, 2:4], xt_d[:, 2:4], 0),
        (xt[:, 4:6], xt_d[:, 4:6], 0),
        (wq[:, 0:3], wq_d[:, 0:3], 0), (wq[:, 3:6], wq_d[:, 3:6], 0),
        (bq[:], bq_d[:], 0),
        (wk[:, 0:3], wk_d[:, 0:3], 1), (wk[:, 3:6], wk_d[:, 3:6], 1),
        (bk[:], bk_d[:], 1),
        (wv[:], wv_d[:], 2), (vb[:], vb_d[:], 2),
        (tri2[:], tri2_d[:], 3), (id16[:], id16_d[:], 3),
        (wo[:], wo_d[:], 4),
    ]
    gtot = [0] * 5
    for a_, b_, gi in loads:
        gtot[gi] += 16
    for a_, b_, gi in loads:
        emit("sync", lambda e, a=a_, b=b_: e.dma_start(out=a, in_=b),
             inc=G[gi], k=16)

    # ================= unified psum pool (banks 0-5 of ps) =================
    bank_war = [None] * 6     # (ctr, val) that frees the bank
    bank_cur = [0]

    def alloc_banks(n):
        # n = 1 or 2 (pair must be even-aligned)
        cur = bank_cur[0]
        if n == 2 and cur % 2 == 1:
            cur += 1
        cur = cur % 6
        banks = [cur, cur + 1] if n == 2 else [cur]
        bank_cur[0] = (banks[-1] + 1) % 6
        for b_ in banks:
            w = bank_war[b_]
            if w is not None:
                wait("pe", w[0], w[1])
        return banks

    def set_war(banks, ctr, val):
        for b_ in banks:
            bank_war[b_] = (ctr, val)

    qk_ready = {}
    v_ready = {}
    exp_ev = {}
    pt_rdy = {}               # chunk u -> (ctr, val) gating AV read of pt
    av_ev = {}
    psz_war = [None, None]    # per head-slot: (ctr, val) of last evac
    rt_ev = {}                # g -> DVE event: recipT cols for pair g ready
    den_dma = {}
    os_i = [0]
    ho_n = [0]

    # ---------------- projection units ----------------
    def proj_q_unit(gp, half, which):
        w_sb, b_sb, dst = (wq, bq, qT) if which == 'q' else (wk, bk, kT)
        bks = alloc_banks(1)
        off = bks[0] * 512
        wait("pe", G[0], gtot[0])
        if which == 'k':
            wait("pe", G[1], gtot[1])
        ev = None
        for dt in range(NT_D):
            ev = emit("pe", lambda e, o=ps[:, off:off + 512],
                      l=w_sb[:, dt, gp * 128:(gp + 1) * 128],
                      r=xt[:, dt, half * 512:(half + 1) * 512],
                      s=(dt == 0), st_=(dt == NT_D - 1):
                      e.matmul(o, l, r, start=s, stop=st_),
                      inc=PEc if dt == NT_D - 1 else None)
        wait("act", PEc, ev)
        cev = emit("act", lambda e, o=dst[:, gp, half * 512:(half + 1) * 512],
                   i=ps[:, off:off + 512], bb=b_sb[:, gp:gp + 1]:
                   e.activation(o, i, Ident, bias=bb), inc=ACTc)
        set_war(bks, ACTc, cev)
        qk_ready[gp] = cev

    def proj_v_unit(pt3, sp):
        # v columns for head-pairs 3*pt3..3*pt3+2 (390 cols), key-tiles 2sp,2sp+1
        bks = alloc_banks(2)
        off = bks[0] * 512
        wait("pe", G[0], gtot[0])
        wait("pe", G[2], gtot[2])
        sts = [2 * sp, 2 * sp + 1]
        ev = None
        for si, st in enumerate(sts):
            o_ap = ps[:, off + 512 * si:off + 512 * si + 390]
            for dt in range(NT_D):
                emit("pe", lambda e, o=o_ap,
                     l=xt[:, dt, st * 128:(st + 1) * 128],
                     r=wv[:, dt, 390 * pt3:390 * pt3 + 390], s=(dt == 0):
                     e.matmul(o, l, r, start=s, stop=False))
            ev = emit("pe", lambda e, o=o_ap,
                      l=ones_b[0:1, 0:128], r=vb[0:1, 390 * pt3:390 * pt3 + 390]:
                      e.matmul(o, l, r, start=False, stop=True),
                      inc=PEc if si == 1 else None)
        wait("dve", PEc, ev)
        src = ps[:, off:off + 1024].rearrange("p (n f) -> p n f", n=2)[:, :, 0:390]
        dst = vA[:, 2 * sp:2 * sp + 2, 390 * pt3:390 * pt3 + 390]
        cev = emit("dve", lambda e, o=dst, i=src: e.tensor_copy(o, i), inc=DVEc)
        set_war(bks, DVEc, cev)
        v_ready[pt3] = cev

    # ---------------- denominator reciprocal, transposed (pair g) ----------
    # recipT[:, 16g + 8*hs + mt] = 1/den[2g+hs, 128*mt + row]; the softmax
    # normalization commutes with W_O, so it's applied as a per-partition
    # scale during head-out evacuation.
    def recipT_chain(g):
        wait("pe", DN, den_dma[g])
        wait("pe", G[3], gtot[3])
        bks = alloc_banks(1)
        off = bks[0] * 512
        tev = emit("pe", lambda e, o=ps[:, off:off + 16],
                   i=den96[0:16, 128 * g:128 * g + 128], ii=id16[:, :]:
                   e.transpose(o, i, ii), inc=PEc)
        wait("dve", PEc, tev)

        def _recip(e, o=recipT[:, 16 * g:16 * g + 16], i=ps[:, off:off + 16]):
            with nc.allow_low_precision(reason="softmax denom recip"):
                return e.reciprocal(o, i)

        rev = emit("dve", _recip, inc=DVEc)
        set_war(bks, DVEc, rev)
        rt_ev[g] = rev

    # ---------------- head-out unit ----------------
    def ho_unit(g, j, tail=False):
        # interleave heads so consecutive units hit different PE row groups
        hs, mt = j % 2, j // 2
        h = 2 * g + hs
        po = hs * 64
        wait("pe", DVEc, rt_ev[g])
        wait("pe", G[4], gtot[4])
        bks = alloc_banks(2)
        off = bks[0] * 512
        emit("pe", lambda e, o=ps[:, off:off + 512],
             l=zT[po:po + 64, g, mt * 128:(mt + 1) * 128],
             r=wo[po:po + 64, g, 0:512]:
             e.matmul(o, l, r, start=True, stop=True))
        ev = emit("pe", lambda e, o=ps[:, off + 512:off + 768],
                  l=zT[po:po + 64, g, mt * 128:(mt + 1) * 128],
                  r=wo[po:po + 64, g, 512:768]:
                  e.matmul(o, l, r, start=True, stop=True), inc=PEc)
        c = os_i[0] % NOS
        os_i[0] += 1
        if tail:
            eng = "act" if j % 2 == 0 else "dve"
        else:
            eng = "act" if ho_n[0] % 3 == 1 else "dve"
        ho_n[0] += 1
        ctr = ACTc if eng == "act" else DVEc
        wait(eng, PEc, ev)
        wait(eng, CH[c], CH[c].n)
        sc = recipT[:, 16 * g + 8 * hs + mt:16 * g + 8 * hs + mt + 1]
        if eng == "act":
            wait(eng, DVEc, rt_ev[g])
            cev = emit(eng, lambda e, o=ostage[c][:, :], i=ps[:, off:off + 768],
                       s=sc: e.activation(o, i, Ident, scale=s), inc=ctr)
        else:
            cev = emit(eng, lambda e, o=ostage[c][:, :], i=ps[:, off:off + 768],
                       s=sc: e.tensor_scalar_mul(o, i, s), inc=ctr)
        set_war(bks, ctr, cev)
        wait("sync", ctr, cev)
        emit("sync", lambda e, o=o12_d[h, mt], i=ostage[c][:, :]:
             e.dma_start(out=o, in_=i), inc=CH[c], k=16)

    # ---------------- attention ----------------
    def s_chunk(g, i, u):
        kt, qoff, c = CHUNKS[i]
        bks = alloc_banks(2)
        off = bks[0] * 512
        wait("pe", ACTc, qk_ready[g])
        ev = None
        for hs in range(2):
            po = hs * 64
            ev = emit("pe", lambda e,
                      o=ps[:, off + hs * 512:off + hs * 512 + c],
                      l=kT[po:po + 64, g, kt * 128:(kt + 1) * 128],
                      r=qT[po:po + 64, g, qoff:qoff + c]:
                      e.matmul(o, l, r, start=True, stop=True),
                      inc=PEc if hs == 1 else None)
        slot = u % NPT
        w = pt_rdy.get(u - NPT)
        if w is not None:
            wait("act", w[0], w[1])
        wait("act", PEc, ev)
        src = ps[:, off:off + 1024].rearrange("p (n f) -> p n f", n=2)[:, :, 0:c]
        dst = pts[slot][:, 0:2 * c].rearrange("p (n f) -> p n f", n=2)
        eev = emit("act", lambda e, o=dst, i=src:
                   e.activation(o, i, Exp, bias=0.0, scale=0.125), inc=ACTc)
        exp_ev[u] = eev
        set_war(bks, ACTc, eev)
        if i in DIAG:
            # multiplicative triangular mask on both heads' diagonal 128-col
            # blocks: pt[:, {0:128, c:c+128}] *= tri
            wait("dve", G[3], gtot[3])
            wait("dve", ACTc, eev)
            ap = pts[slot][:, 0:2 * c].rearrange(
                "p (n f) -> p n f", n=2)[:, :, 0:128]
            mev = emit("dve", lambda e, o=ap,
                       m=tri2[:, :].rearrange("p (n f) -> p n f", n=2):
                       e.tensor_mul(o, o, m), inc=DVEc)
            pt_rdy[u] = (DVEc, mev)
        else:
            pt_rdy[u] = (ACTc, eev)

    def av_chunk(g, i, u):
        kt, qoff, c = CHUNKS[i]
        qo = qoff - 512 * (i >= 4)
        slot = u % NPT
        ctr, v = pt_rdy[u]
        wait("pe", ctr, v)
        wait("pe", DVEc, v_ready[g // 3])
        start = i in (0, 4)
        stop = i in (3, 11)
        ev = None
        for hs in range(2):
            h = 2 * g + hs
            if start and psz_war[hs] is not None:
                wait("pe", psz_war[hs][0], psz_war[hs][1])
            ev = emit("pe", lambda e, o=psZ[hs][0:65, qo:qo + c],
                      l=vA[:, kt, 65 * h:65 * h + 65],
                      r=pts[slot][:, hs * c:hs * c + c],
                      s=start, st_=stop:
                      e.matmul(o, l, r, start=s, stop=st_),
                      inc=PEc if hs == 1 else None)
        av_ev[u] = ev

    def evac_half(g, half, u_last):
        # head0 via DVE, head1 via ACT (parallel evacuation chains)
        for hs, eng, ctr in ((0, "dve", DVEc), (1, "act", ACTc)):
            h = 2 * g + hs
            po = hs * 64
            wait(eng, PEc, av_ev[u_last])
            zt_ap = zT[po:po + 64, g, half * 512:(half + 1) * 512]
            dn_ap = den_sb[0:1, 1024 * h + half * 512:
                           1024 * h + (half + 1) * 512]
            if eng == "dve":
                emit(eng, lambda e, o=zt_ap, i=psZ[hs][0:64, :]:
                     e.tensor_copy(o, i))
                dev = emit(eng, lambda e, o=dn_ap, i=psZ[hs][64:65, :]:
                           e.tensor_copy(o, i), inc=ctr)
            else:
                emit(eng, lambda e, o=zt_ap, i=psZ[hs][0:64, :]:
                     e.copy(o, i))
                dev = emit(eng, lambda e, o=dn_ap, i=psZ[hs][64:65, :]:
                           e.copy(o, i), inc=ctr)
            psz_war[hs] = (ctr, dev)
        if half == 1:
            for hs in range(2):
                h = 2 * g + hs
                wait("sync", psz_war[hs][0], psz_war[hs][1])
                emit("sync", lambda e,
                     o=den96[8 * hs:8 * hs + 8, 128 * g:128 * g + 128],
                     i=den_sb[0:1, 1024 * h:1024 * h + 1024]:
                     e.dma_start(out=o, in_=i), inc=DN, k=16)
            den_dma[g] = DN.n

    # ================= emission =================
    wait("pe", DVEc, ev_ones)
    # keep the PE activity monitor busy during the input-DMA window so the
    # clock gate is released before real work starts
    for _ in range(8):
        emit("pe", lambda e, o=psZ[0][:, 0:512],
             l=ones_b[0:1, 0:128], r=ones_b[0:1, 0:512]:
             e.matmul(o, l, r, start=True, stop=True))
    proj_q_unit(0, 0, 'q')
    proj_q_unit(0, 1, 'q')
    proj_q_unit(0, 0, 'k')
    proj_q_unit(0, 1, 'k')
    for sp in range(4):
        proj_v_unit(0, sp)

    def fillers_for(g):
        f = {i: [] for i in range(12)}
        if g < 5:
            gp = g + 1
            f[0].append(lambda: proj_q_unit(gp, 0, 'q'))
            f[1].append(lambda: proj_q_unit(gp, 1, 'q'))
            f[2].append(lambda: proj_q_unit(gp, 0, 'k'))
            f[3].append(lambda: proj_q_unit(gp, 1, 'k'))
        if g == 0:
            for sp in range(4):
                f[4 + 2 * sp].append(lambda sp=sp: proj_v_unit(1, sp))
        if g >= 1:
            gm = g - 1
            f[1].append(lambda: recipT_chain(gm))
            for j in range(16):
                f[2 + (j // 2)].append(lambda j=j: ho_unit(gm, j))
        return f

    for g in range(6):
        f = fillers_for(g)
        u0 = 12 * g
        s_chunk(g, 0, u0)
        s_chunk(g, 1, u0 + 1)
        for i in range(12):
            if i + 2 < 12:
                s_chunk(g, i + 2, u0 + i + 2)
            for th in f[i]:
                th()
            av_chunk(g, i, u0 + i)
            if i == 3:
                evac_half(g, 0, u0 + 3)
            if i == 11:
                evac_half(g, 1, u0 + 11)

    recipT_chain(5)
    for j in range(16):
        ho_unit(5, j, tail=True)

    for c in range(NOS):
        wait("sync", CH[c], CH[c].n)

    with nc.Block() as block:
        @block.tensor
        def _(e):
            for fn in prog["pe"]:
                fn(e)

        @block.scalar
        def _(e):
            for fn in prog["act"]:
                fn(e)

        @block.vector
        def _(e):
            for fn in prog["dve"]:
                fn(e)

        @block.sync
        def _(e):
            for fn in prog["sync"]:
                fn(e)

    ctx.close()
    return nc


def _prep_shared(inputs):
    WQ = np.asarray(inputs["W_Q"], np.float32)
    bQ = np.asarray(inputs["b_Q"], np.float32)
    WK = np.asarray(inputs["W_K"], np.float32)
    bK = np.asarray(inputs["b_K"], np.float32)
    WV = np.asarray(inputs["W_V"], np.float32)
    bV = np.asarray(inputs["b_V"], np.float32)
    WO = np.asarray(inputs["W_O"], np.float32)

    def to_tiles(w):
        return np.ascontiguousarray(
            w.reshape(NT_D, 128, w.shape[1]).transpose(1, 0, 2))

    wq = to_tiles(WQ.transpose(1, 0, 2).reshape(768, 768)).astype(BF16)
    wk = to_tiles(WK.transpose(1, 0, 2).reshape(768, 768)).astype(BF16)
    wv_aug = np.zeros((768, 780), np.float32)
    vb_row = np.zeros((1, 780), np.float32)
    for h in range(N_HEADS):
        wv_aug[:, 65 * h:65 * h + 64] = WV[h]
        vb_row[0, 65 * h:65 * h + 64] = bV[h]
        vb_row[0, 65 * h + 64] = 1.0
    wv = to_tiles(wv_aug).astype(BF16)
    wo = to_tiles(WO.reshape(768, 768)).astype(BF16)
    bq_r = np.ascontiguousarray(bQ.reshape(NT_D, 128).T).astype(np.float32)
    bk_r = np.ascontiguousarray(bK.reshape(NT_D, 128).T).astype(np.float32)

    r = np.arange(128)[:, None]
    c = np.arange(128)[None, :]
    tri = (r <= c).astype(np.float32)
    tri2 = np.concatenate([tri, tri], axis=1)

    return {
        "wq": wq, "wk": wk, "wv": wv, "wo": wo,
        "bq": bq_r, "bk": bk_r, "vb": vb_row.astype(BF16),
        "tri2": tri2.astype(BF16), "id16": np.eye(16, dtype=np.float32),
    }


def _prep_in_maps(inputs):
    nrp = np.asarray(inputs["normalized_resid_pre"], np.float32)
    alt = np.asarray(inputs["alt_normalized_resid_pre"], np.float32)
    shared = _prep_shared(inputs)
    in_maps = []
    for b in range(B):
        for p in range(2):
            x = nrp[b, 0] if p == 0 else alt[b]
            xt = np.ascontiguousarray(
                x.T.reshape(NT_D, 128, S).transpose(1, 0, 2))
            m = dict(shared)
            m["xt"] = xt.astype(BF16)
            in_maps.append(m)
    return in_maps


def _assemble(results, inputs):
    bO = np.asarray(inputs["b_O"], np.float32)
    out = np.empty((B, 14, S, D_MODEL), np.float32)
    for b in range(B):
        o0 = results[2 * b]["o12"].reshape(N_HEADS, S, D_MODEL).astype(np.float32)
        o1 = results[2 * b + 1]["o12"].reshape(N_HEADS, S, D_MODEL).astype(np.float32)
        ch0 = o0.sum(0) + bO
        ch1 = o1.sum(0) + bO
        out[b, 0] = ch0
        out[b, 1] = ch1
        out[b, 2:] = ch1[None] - o1 + o0
    return out


def _ensure_profile_hook():
    import sys
    import types

    try:
        from antenv.axon_hooks import get_axon_ntff_profile_hook  # noqa: F401
        return True
    except ImportError:
        pass
    try:
        from trn_agent_boot.trn_boot import _ntff_profile_via_ctypes

        hook = _ntff_profile_via_ctypes("/opt/axon/libaxon_pjrt.so")
        if hook is None:
            return False
        mod = types.ModuleType("antenv.axon_hooks")
        state = {"hook": hook}
        mod.set_axon_ntff_profile_hook = lambda h: state.update(hook=h)
        mod.get_axon_ntff_profile_hook = lambda: state["hook"]
        sys.modules["antenv.axon_hooks"] = mod
        import antenv

        antenv.axon_hooks = mod
        return True
    except Exception:
        return False


def kernel(**inputs):
    global LAST_EXEC_NS, _GRAPH
    from concourse.bass_utils import run_bass_kernel_spmd

    if _GRAPH is None:
        _GRAPH = _build_graph()
    nc = _GRAPH
    in_maps = _prep_in_maps(inputs)
    trace = os.environ.get("KERNEL_PROFILE", "0") == "1"
    if trace:
        trace = _ensure_profile_hook()
    res = run_bass_kernel_spmd(nc, in_maps, list(range(8)), trace=trace)
    LAST_EXEC_NS = res.exec_time_ns
    return _assemble(res.results, inputs)
